# revision 12
# baseline (speedup 1.0000x reference)
"""Trainium2 Bass kernel for nn_Block_Order_Aware_Filtering_1_to_1.

Reference math (B=32, C=128, N=4096, M=512, L=6):
  xs = x[..., 0]                                            [B, C, N]
  Spool = softmax_n(W_pool @ xs)                            [B, M, N]
  h     = einsum('bcn,bmn->bmc', xs, Spool)                 [B, M, C]
  6x:  y = Wf[l] @ h ; BN over (B, C) ; h = relu(yn + h)
  ul  = W_unpool @ xs + b_unpool                            [B, M, N]
  Sun = softmax_m(ul)
  out = einsum('bcm,bmn->bcn', h^T, Sun)[..., None]

Sharding: data-parallel over B across 8 cores (4 batch each); params
replicated; BN batch stats exchanged with one small AllReduce per layer
(sum(y), sum(y^2) per channel -> exact global-batch parity).

Per-core pipeline (all matmuls f32r = 1 cyc/row at >=256 moving cols):
  pool:   logitsT[n,m] tiles = xs_chunk^T @ W_poolT ; E=exp (no max-sub:
          |logit| <~ 4 so exp is safe and exactly equal in infinite
          precision to the max-subtracted softmax); hT[c,m] += xsT_chunk^T
          @ E_chunk ; Z[1,m] += ones^T @ E_chunk ; h = (hT * (1/Z))^T
  filter: y[o, (b,c)] = WfT_chunk^T @ h_chunk ; local sums -> AllReduce
          -> mean/var -> h = relu(a*y + b + h)
  unpool: ul[m,n] = W_unpoolT_chunk^T @ xs ; E = exp(ul + b_un);
          Z[1,n] += ones^T @ E_chunk ; out[c,n] += h_chunk^T @ E_chunk ;
          out *= bcast(1/Z) via PE outer-product ; DMA out.
"""

import os

import numpy as np

import concourse.bass as bass
import concourse.tile as tile
from concourse import bacc, mybir

F32 = mybir.dt.float32
F32R = mybir.dt.float32r
AF = mybir.ActivationFunctionType
ALU = mybir.AluOpType

B, C, N, M, L = 32, 128, 4096, 512, 6
NCORES = 8
BL = B // NCORES          # 4 batch items per core
NT = N // 128             # 32 n-chunks of 128
MT = M // 128             # 4 m/o-chunks of 128
NTile = N // 512          # 8 n-tiles of 512
EPS = 1e-5
USE_F32R = True
# debug bisection knobs (affect the built program - clear _CACHE to rebuild)
DBG_LAYERS = int(os.environ.get("BASSK_LAYERS", str(L)))
DBG_UNPOOL = os.environ.get("BASSK_UNPOOL", "1") == "1"
DBG_POOL = os.environ.get("BASSK_POOL", "1") == "1"
DBG_NOAR = os.environ.get("BASSK_NOAR", "0") == "1"


def _mm(ap):
    """Matmul operands are natively float32r (producers round on write,
    as the walrus BIR verifier requires); this is a no-op hook kept for
    switching the kernel back to plain fp32."""
    return ap


def _pool_phase(nc, tc, x_d, xsp, wp_sb, ident, ones_col, ones_row, h0):
    with (
        tc.tile_pool(name="psA_log", bufs=2, space="PSUM") as psA_log,
        tc.tile_pool(name="psA_acc", bufs=1, space="PSUM") as psA_acc,
        tc.tile_pool(name="psA_z", bufs=1, space="PSUM") as psA_z,
        tc.tile_pool(name="psA_tr", bufs=2, space="PSUM") as psA_tr,
        tc.tile_pool(name="xsT", bufs=1) as xsTp,
        tc.tile_pool(name="ET", bufs=1) as ETp,
        tc.tile_pool(name="poolsc", bufs=2) as scp,
    ):
        for b in range(BL):
            xs_sb = xsp.tile([C, N], F32R, name=f"xs_{b}", tag="xs")
            nc.sync.dma_start(out=xs_sb, in_=x_d[b])
            xsT_sb = xsTp.tile([128, NT * C], F32R, name=f"xsT_{b}", tag="xsT")
            ET_sb = ETp.tile([128, NT * M], F32R, name=f"ET_{b}", tag="ET")
            ps_hT = psA_acc.tile([128, M], F32, name=f"hT_{b}", tag="hT")
            ps_z = psA_z.tile([128, M], F32, name=f"z_{b}", tag="z")

            # software-pipelined: group g = 2 n-chunks; consume group g-1
            # (transpose + h/z accumulation) while group g's logits+exp run.
            def consume(g):
                for k in range(2):
                    ni = 2 * g + k
                    ps_tr = psA_tr.tile([128, 128], F32R, name=f"tr_{b}_{ni}",
                                        tag="tr")
                    nc.tensor.transpose(ps_tr, xs_sb[:, ni * 128:(ni + 1) * 128],
                                        ident)
                    nc.vector.tensor_copy(out=xsT_sb[:, ni * C:(ni + 1) * C],
                                          in_=ps_tr)
                    nc.tensor.matmul(
                        ps_hT, _mm(xsT_sb[:, ni * C:(ni + 1) * C]),
                        _mm(ET_sb[:, ni * M:(ni + 1) * M]),
                        start=(ni == 0), stop=(ni == NT - 1))
                    nc.tensor.matmul(
                        ps_z[0:1, :], _mm(ones_col),
                        _mm(ET_sb[:, ni * M:(ni + 1) * M]),
                        start=(ni == 0), stop=(ni == NT - 1))

            for g in range(NT // 2):
                ps_log = psA_log.tile([128, 2 * M], F32, name=f"log_{b}_{g}",
                                      tag="log")
                for k in range(2):
                    ni = 2 * g + k
                    nc.tensor.matmul(ps_log[:, k * M:(k + 1) * M],
                                     _mm(xs_sb[:, ni * 128:(ni + 1) * 128]),
                                     _mm(wp_sb), start=True, stop=True)
                nc.scalar.activation(out=ET_sb[:, g * 2 * M:(g + 1) * 2 * M],
                                     in_=ps_log, func=AF.Exp)
                if g >= 1:
                    consume(g - 1)
            consume(NT // 2 - 1)

            # finish batch b: h = (hT * 1/Z)^T, scattered into h0 chunks
            rz = scp.tile([1, M], F32R, name=f"rz_{b}", tag="rz")
            with nc.allow_low_precision("1/Z rounds to f32r for the PE broadcast"):
                nc.vector.reciprocal(out=rz, in_=ps_z[0:1, :])
            ps_rp = psA_tr.tile([128, M], F32, name=f"rp_{b}", tag="tr",
                                padded_shape=None)
            nc.tensor.matmul(ps_rp, _mm(ones_row), _mm(rz), start=True,
                             stop=True)
            rp_sb = scp.tile([128, M], F32, name=f"rp_sb_{b}", tag="rp_sb")
            nc.vector.tensor_copy(out=rp_sb, in_=ps_rp)
            hTs = scp.tile([128, M], F32R, name=f"hTs_{b}", tag="hTs")
            nc.vector.tensor_mul(hTs, ps_hT, rp_sb)
            for mi in range(MT):
                ps_h = psA_tr.tile([128, 128], F32R, name=f"h_{b}_{mi}",
                                   tag="tr")
                nc.tensor.transpose(ps_h, hTs[:, mi * 128:(mi + 1) * 128],
                                    ident)
                nc.vector.tensor_copy(out=h0[mi][:, b * C:(b + 1) * C],
                                      in_=ps_h)


def _filter_phase(nc, tc, wfT_d, gb_sb, bb_sb, eps_sb, hp, h_cur, dramp):
    inv_bc = 1.0 / float(B * C)
    with (
        tc.tile_pool(name="psB_y", bufs=4, space="PSUM") as psB_y,
        tc.tile_pool(name="wf", bufs=8) as wfp,
        tc.tile_pool(name="ysb", bufs=4) as ysbp,
        tc.tile_pool(name="fsc", bufs=2) as fscp,
        tc.tile_pool(name="fst", bufs=2) as fstp,
    ):
        for l in range(DBG_LAYERS):
            wf_sb = []
            for mi in range(MT):
                w = wfp.tile([128, M], F32R, name=f"wf_{l}_{mi}", tag="wf")
                nc.sync.dma_start(out=w, in_=wfT_d[l, mi * 128:(mi + 1) * 128, :])
                wf_sb.append(w)

            stats = fstp.tile([128, 2 * MT], F32, name=f"st_{l}", tag="st")
            y_sb = []
            for oi in range(MT):
                ps_y = psB_y.tile([128, BL * C], F32, name=f"y_{l}_{oi}",
                                  tag="y")
                for mi in range(MT):
                    nc.tensor.matmul(
                        ps_y, _mm(wf_sb[mi][:, oi * 128:(oi + 1) * 128]),
                        _mm(h_cur[mi]), start=(mi == 0), stop=(mi == MT - 1))
                y = ysbp.tile([128, BL * C], F32, name=f"ysb_{l}_{oi}",
                              tag="ysb")
                # evacuate PSUM->SBUF and accumulate sum(y) in one ACT op
                nc.scalar.activation(out=y, in_=ps_y, func=AF.Copy,
                                     accum_out=stats[:, oi:oi + 1])
                sq = fscp.tile([128, BL * C], F32, name=f"sq_{l}_{oi}",
                               tag="sq")
                # (y * 1.0) * y with per-partition sum accumulation; DVE's
                # tensor_tensor_reduce crashes this runtime, this doesn't.
                nc.vector.scalar_tensor_tensor(
                    out=sq, in0=y, scalar=1.0, in1=y,
                    op0=ALU.mult, op1=ALU.mult,
                    accum_out=stats[:, MT + oi:MT + oi + 1])
                y_sb.append(y)

            # exchange partial sums -> exact global-batch BN stats
            gst = fstp.tile([128, 2 * MT], F32, name=f"gst_{l}", tag="st")
            if DBG_NOAR:
                nc.vector.tensor_scalar_mul(gst, stats, float(NCORES))
            else:
                st_in = dramp.tile([128, 2 * MT], F32, name=f"sti_{l}",
                                   tag=f"sti{l}", bufs=1)
                st_out = dramp.tile([128, 2 * MT], F32, name=f"sto_{l}",
                                    tag=f"sto{l}", bufs=1, addr_space="Shared")
                nc.sync.dma_start(out=st_in, in_=stats)
                nc.gpsimd.collective_compute(
                    "AllReduce", ALU.add,
                    replica_groups=[list(range(NCORES))],
                    ins=[st_in.opt()], outs=[st_out.opt()])
                nc.sync.dma_start(out=gst, in_=st_out)

            mean = fstp.tile([128, MT], F32, name=f"mean_{l}", tag="mean")
            nc.vector.tensor_scalar_mul(mean, gst[:, 0:MT], inv_bc)
            msq = fstp.tile([128, MT], F32, name=f"msq_{l}", tag="msq")
            nc.vector.tensor_scalar_mul(msq, gst[:, MT:2 * MT], inv_bc)
            m2 = fstp.tile([128, MT], F32, name=f"m2_{l}", tag="m2")
            nc.vector.tensor_mul(m2, mean, mean)
            var = fstp.tile([128, MT], F32, name=f"var_{l}", tag="var")
            nc.vector.scalar_tensor_tensor(out=var, in0=m2, scalar=-1.0,
                                           in1=msq, op0=ALU.mult, op1=ALU.add)
            std = fstp.tile([128, MT], F32, name=f"std_{l}", tag="std")
            nc.scalar.activation(out=std, in_=var, func=AF.Sqrt, bias=eps_sb)
            rstd = fstp.tile([128, MT], F32, name=f"rstd_{l}", tag="rstd")
            nc.vector.reciprocal(out=rstd, in_=std)
            a_t = fstp.tile([128, MT], F32, name=f"a_{l}", tag="a")
            nc.vector.tensor_mul(a_t, gb_sb[:, l * MT:(l + 1) * MT], rstd)
            ma = fstp.tile([128, MT], F32, name=f"ma_{l}", tag="ma")
            nc.vector.tensor_mul(ma, mean, a_t)
            b_t = fstp.tile([128, MT], F32, name=f"b_{l}", tag="b")
            nc.vector.scalar_tensor_tensor(out=b_t, in0=ma, scalar=-1.0,
                                           in1=bb_sb[:, l * MT:(l + 1) * MT],
                                           op0=ALU.mult, op1=ALU.add)

            h_next = []
            for oi in range(MT):
                tmp = fscp.tile([128, BL * C], F32, name=f"tmp_{l}_{oi}",
                                tag="sq")
                nc.vector.scalar_tensor_tensor(
                    out=tmp, in0=y_sb[oi], scalar=a_t[:, oi:oi + 1],
                    in1=h_cur[oi], op0=ALU.mult, op1=ALU.add)
                hn = hp.tile([128, BL * C], F32R, name=f"h_{l + 1}_{oi}",
                             tag="h")
                nc.scalar.activation(out=hn, in_=tmp, func=AF.Relu,
                                     bias=b_t[:, oi:oi + 1])
                h_next.append(hn)
            h_cur = h_next
    return h_cur


def _unpool_phase(nc, tc, x_d, out_d, xsp, wu_sb, bu_sb, ones_col, ones_row,
                  h_fin):
    with (
        tc.tile_pool(name="psC_ul", bufs=2, space="PSUM") as psC_ul,
        tc.tile_pool(name="psC_out", bufs=2, space="PSUM") as psC_out,
        tc.tile_pool(name="psC_z", bufs=1, space="PSUM") as psC_z,
        tc.tile_pool(name="psC_r", bufs=1, space="PSUM") as psC_r,
        tc.tile_pool(name="EU", bufs=6) as EUp,
        tc.tile_pool(name="outsb", bufs=3) as outp,
        tc.tile_pool(name="usc", bufs=2) as uscp,
    ):
        for b in range(BL):
            xs_sb = xsp.tile([C, N], F32R, name=f"xsu_{b}", tag="xs")
            nc.sync.dma_start(out=xs_sb, in_=x_d[b])
            for nj in range(NTile // 2):       # pairs of 512-wide n-tiles
                eu = []
                for mi in range(MT):
                    ps_ul = psC_ul.tile([128, 1024], F32,
                                        name=f"ul_{b}_{nj}_{mi}", tag="ul")
                    for k in range(2):
                        nc.tensor.matmul(
                            ps_ul[:, k * 512:(k + 1) * 512],
                            _mm(wu_sb[:, mi * 128:(mi + 1) * 128]),
                            _mm(xs_sb[:, (2 * nj + k) * 512:
                                      (2 * nj + k + 1) * 512]),
                            start=True, stop=True)
                    e = EUp.tile([128, 1024], F32R, name=f"eu_{b}_{nj}_{mi}",
                                 tag="eu")
                    nc.scalar.activation(out=e, in_=ps_ul, func=AF.Exp,
                                         bias=bu_sb[:, mi:mi + 1])
                    eu.append(e)
                for k in range(2):
                    ni = 2 * nj + k
                    ps_zu = psC_z.tile([128, 512], F32, name=f"zu_{b}_{ni}",
                                       tag="zu")
                    for mi in range(MT):
                        nc.tensor.matmul(
                            ps_zu[0:1, :], _mm(ones_col),
                            _mm(eu[mi][:, k * 512:(k + 1) * 512]),
                            start=(mi == 0), stop=(mi == MT - 1))
                    rzu = uscp.tile([1, 512], F32R, name=f"rzu_{b}_{ni}",
                                    tag="rzu")
                    with nc.allow_low_precision("1/Z rounds to f32r for the PE broadcast"):
                        nc.vector.reciprocal(out=rzu, in_=ps_zu[0:1, :])
                    ps_r = psC_r.tile([128, 512], F32, name=f"rb_{b}_{ni}",
                                      tag="rb")
                    nc.tensor.matmul(ps_r, _mm(ones_row), _mm(rzu),
                                     start=True, stop=True)
                    ps_o = psC_out.tile([128, 512], F32, name=f"o_{b}_{ni}",
                                        tag="o")
                    for mi in range(MT):
                        nc.tensor.matmul(
                            ps_o, _mm(h_fin[mi][:, b * C:(b + 1) * C]),
                            _mm(eu[mi][:, k * 512:(k + 1) * 512]),
                            start=(mi == 0), stop=(mi == MT - 1))
                    r_sb = uscp.tile([128, 512], F32, name=f"rsb_{b}_{ni}",
                                     tag="rsb")
                    nc.vector.tensor_copy(out=r_sb, in_=ps_r)
                    o_sb = outp.tile([128, 512], F32, name=f"os_{b}_{ni}",
                                     tag="os")
                    nc.vector.tensor_mul(o_sb, ps_o, r_sb)
                    nc.sync.dma_start(
                        out=out_d[b, :, ni * 512:(ni + 1) * 512], in_=o_sb)


def _kernel_body(nc, tc, x_d, wpT_d, wfT_d, gamma_d, beta_d, wuT_d, bu_d,
                 ident_d, ones_d, out_d):
    with (
        tc.tile_pool(name="const", bufs=1) as constp,
        tc.tile_pool(name="xs", bufs=2) as xsp,
        tc.tile_pool(name="h", bufs=8) as hp,
        tc.tile_pool(name="dram", bufs=2, space="DRAM") as dramp,
    ):
        ident = constp.tile([128, 128], F32R)
        nc.sync.dma_start(out=ident, in_=ident_d)
        ones_col = constp.tile([128, 1], F32R)
        nc.sync.dma_start(out=ones_col, in_=ones_d[:, 0:1])
        ones_row = constp.tile([1, 128], F32R)
        nc.sync.dma_start(out=ones_row, in_=ones_d[0:1, :])
        wp_sb = constp.tile([C, M], F32R)
        nc.sync.dma_start(out=wp_sb, in_=wpT_d)
        wu_sb = constp.tile([C, M], F32R)
        nc.sync.dma_start(out=wu_sb, in_=wuT_d)
        gb_sb = constp.tile([128, L * MT], F32)
        nc.sync.dma_start(out=gb_sb.rearrange("p (l o) -> p l o", l=L),
                          in_=gamma_d.rearrange("l (o p) -> p l o", p=128))
        bb_sb = constp.tile([128, L * MT], F32)
        nc.sync.dma_start(out=bb_sb.rearrange("p (l o) -> p l o", l=L),
                          in_=beta_d.rearrange("l (o p) -> p l o", p=128))
        eps_sb = constp.tile([128, 1], F32)
        nc.vector.memset(eps_sb, EPS)
        bu_sb = constp.tile([128, MT], F32)
        nc.sync.dma_start(out=bu_sb, in_=bu_d.rearrange("(o p) -> p o", p=128))

        h0 = [hp.tile([128, BL * C], F32R, name=f"h_0_{mi}", tag="h")
              for mi in range(MT)]
        if DBG_POOL:
            _pool_phase(nc, tc, x_d, xsp, wp_sb, ident, ones_col, ones_row, h0)
        else:
            for mi in range(MT):
                nc.sync.dma_start(out=h0[mi],
                                  in_=wpT_d.rearrange("c m -> c m")[0:128, 0:BL * C])
        h_fin = _filter_phase(nc, tc, wfT_d, gb_sb, bb_sb, eps_sb, hp, h0, dramp)
        if DBG_UNPOOL:
            _unpool_phase(nc, tc, x_d, out_d, xsp, wu_sb, bu_sb, ones_col,
                          ones_row, h_fin)
        else:
            o_sb = constp.tile([128, 512], F32)
            nc.vector.tensor_copy(out=o_sb, in_=h_fin[0])
            nc.sync.dma_start(out=out_d[0, :, 0:512], in_=o_sb)


_CACHE = {}


def build():
    if "nc" in _CACHE:
        return _CACHE["nc"]
    nc = bacc.Bacc("TRN2", target_bir_lowering=False, debug=False,
                   num_devices=NCORES)
    x_d = nc.dram_tensor("x", [BL, C, N], F32R, kind="ExternalInput").ap()
    wpT_d = nc.dram_tensor("w_pool_t", [C, M], F32R, kind="ExternalInput").ap()
    wfT_d = nc.dram_tensor("wf_t", [L, M, M], F32R, kind="ExternalInput").ap()
    gamma_d = nc.dram_tensor("gamma", [L, M], F32, kind="ExternalInput").ap()
    beta_d = nc.dram_tensor("beta", [L, M], F32, kind="ExternalInput").ap()
    wuT_d = nc.dram_tensor("w_unpool_t", [C, M], F32R,
                           kind="ExternalInput").ap()
    bu_d = nc.dram_tensor("b_unpool", [M], F32, kind="ExternalInput").ap()
    ident_d = nc.dram_tensor("ident", [128, 128], F32R,
                             kind="ExternalInput").ap()
    ones_d = nc.dram_tensor("ones", [128, 128], F32R,
                            kind="ExternalInput").ap()
    out_d = nc.dram_tensor("out", [BL, C, N], F32, kind="ExternalOutput").ap()

    with tile.TileContext(nc) as tc:
        _kernel_body(nc, tc, x_d, wpT_d, wfT_d, gamma_d, beta_d, wuT_d, bu_d,
                     ident_d, ones_d, out_d)
    nc.compile()
    _CACHE["nc"] = nc
    return nc


def make_in_maps(x, W_pool, Wf, gamma, beta, W_unpool, b_unpool):
    xs = np.ascontiguousarray(np.asarray(x, dtype=np.float32)[..., 0])
    shards = xs.reshape(NCORES, BL, C, N)
    wpT = np.ascontiguousarray(np.asarray(W_pool, np.float32).T)
    wfT = np.ascontiguousarray(
        np.asarray(Wf, np.float32).transpose(0, 2, 1))
    wuT = np.ascontiguousarray(np.asarray(W_unpool, np.float32).T)
    common = {
        "w_pool_t": wpT, "wf_t": wfT,
        "gamma": np.ascontiguousarray(np.asarray(gamma, np.float32)),
        "beta": np.ascontiguousarray(np.asarray(beta, np.float32)),
        "w_unpool_t": wuT,
        "b_unpool": np.ascontiguousarray(np.asarray(b_unpool, np.float32)),
        "ident": np.eye(128, dtype=np.float32),
        "ones": np.ones((128, 128), dtype=np.float32),
    }
    return [{"x": np.ascontiguousarray(shards[i]), **common}
            for i in range(NCORES)]


LAST_RESULTS = None


def kernel(x, W_pool, Wf, gamma, beta, W_unpool, b_unpool, trace=False):
    global LAST_RESULTS
    from concourse.bass_utils import run_bass_kernel_spmd
    nc = build()
    in_maps = make_in_maps(x, W_pool, Wf, gamma, beta, W_unpool, b_unpool)
    res = run_bass_kernel_spmd(nc, in_maps, core_ids=list(range(NCORES)),
                               trace=trace)
    LAST_RESULTS = res
    out = np.concatenate([res.results[i]["out"] for i in range(NCORES)],
                         axis=0)
    return out.reshape(B, C, N, 1)
